# revision 14
# baseline (speedup 1.0000x reference)
"""Trainium2 Bass kernel for conditional graph-attention generation.

Math (per batch b):
    hq = query @ Wq + bq                      [256]
    hk = key @ Wq + bq                        [N, 256]
    hv = value @ Wv + bv                      [N, 256]
    e_pre[n,h] = sum_d a[d]*lrelu(hq+hk)[n, h*64+d]
    e = softmax_n(e_pre)                      [N, 4]
    scored[h,:] = sum_n e[n,h]*hv[n,h*64:(h+1)*64]
    h' = relu(concat_h scored)                [256]

Device strategy (8 cores, data-parallel over batch, 4 batches/core):
    - hq+hk = key@Wq + (query@Wq + 2*bq); the per-batch vector qb is host-folded.
    - key tiles [128n, 256qd] are DMA-cast to bf16, PE-transposed to [qd, n],
      then hk' = Wq.T @ keyT on the PE (bf16, fp32 accum).
    - ScalarE computes lrelu(hk' + qb) in one pass (per-partition bias).
    - Per-head a-dot is a PE matmul with a block-diagonal `a` as rhs,
      accumulating e_pre[n, (T,h)] slices into one persistent PSUM tile.
    - One Exp per batch gives unnormalized weights w; value weighted-sum is a
      fp32r PE matmul with w column slices as stationary operand.
    - scored = sum_n w*value uses sum_n e = 1:  scored = (s/tot)@Wv + bv with
      s = sum_n w*value, tot = sum_n w -- both normalization and the tiny
      [4,256]@[256,64] Wv projection are done on host.
"""

import numpy as np
import ml_dtypes

B, N, QD, VD = 32, 8192, 256, 256
H, D = 4, 64
HD = H * D  # 256
NEG_SLOPE = 0.2
NCORES = 8
BPC = B // NCORES          # batches per core
TILES = N // 128           # 64 n-tiles per batch
GROUPS = 8                 # groups per batch
GT = TILES // GROUPS       # 8 tiles per group (1024 nodes)

_CACHE = {}


def _build_program(dma_cast=True, act_mode="lrelu", reps=1):
    import concourse.tile as tile
    from concourse import bacc, mybir

    f32 = mybir.dt.float32
    bf16 = mybir.dt.bfloat16
    f32r = mybir.dt.float32r
    AF = mybir.ActivationFunctionType

    nc = bacc.Bacc(
        "TRN2", target_bir_lowering=False, debug=False, num_devices=NCORES
    )

    key_d = nc.dram_tensor("key_c", [BPC, N, QD], f32, kind="ExternalInput")
    val_d = nc.dram_tensor("val_c", [BPC, N, VD], f32r, kind="ExternalInput")
    # qb[p, b*2+m] = (query[b]@Wq + 2*bq)[m*128+p]
    qb_d = nc.dram_tensor("qb_c", [128, BPC * 2], f32, kind="ExternalInput")
    # wq[p, (c*2+m)*128+j] = Wq[c*128+p, m*128+j]   (bf16)
    wq_d = nc.dram_tensor("wq_c", [128, 512], bf16, kind="ExternalInput")
    # ablk[p, m*4+h] = a[d] if m*128+p == h*64+d else 0   (bf16)
    ablk_d = nc.dram_tensor("ablk_c", [128, 8], bf16, kind="ExternalInput")
    ident_d = nc.dram_tensor("ident_c", [128, 128], bf16, kind="ExternalInput")
    if act_mode == "abs":
        # cq06[p, c*4+h] = (Wq @ (0.6 * ablk))[c*128+p, h]
        cq06_d = nc.dram_tensor("cq06_c", [128, 8], bf16, kind="ExternalInput")

    e_d = nc.dram_tensor("e_c", [BPC, N, H], f32r, kind="ExternalOutput")
    s_d = nc.dram_tensor("s_c", [BPC, H, VD], f32, kind="ExternalOutput")

    with tile.TileContext(nc) as tc:
        with (
            tc.tile_pool(name="consts", bufs=1) as consts,
            tc.tile_pool(name="kload", bufs=3) as kload,
            tc.tile_pool(name="keyt", bufs=4) as keyt,
            tc.tile_pool(name="tlr", bufs=4) as tlr,
            tc.tile_pool(name="vload", bufs=3) as vload,
            tc.tile_pool(name="wbuf", bufs=2) as wbuf,
            tc.tile_pool(name="sout", bufs=2) as sout,
            tc.tile_pool(name="ps_t", bufs=3, space="PSUM") as ps_t,
            tc.tile_pool(name="ps_hk", bufs=2, space="PSUM") as ps_hk,
            tc.tile_pool(name="ps_e", bufs=2, space="PSUM") as ps_e,
            tc.tile_pool(name="ps_sc", bufs=1, space="PSUM") as ps_sc,
        ):
            qb_sb = consts.tile([128, BPC * 2], f32)
            nc.sync.dma_start(out=qb_sb[:], in_=qb_d[:])
            wq_sb = consts.tile([128, 512], bf16)
            nc.sync.dma_start(out=wq_sb[:], in_=wq_d[:])
            ablk_sb = consts.tile([128, 8], bf16)
            nc.sync.dma_start(out=ablk_sb[:], in_=ablk_d[:])
            ident_sb = consts.tile([128, 128], bf16)
            nc.sync.dma_start(out=ident_sb[:], in_=ident_d[:])
            if act_mode == "abs":
                cq06_sb = consts.tile([128, 8], bf16)
                nc.sync.dma_start(out=cq06_sb[:], in_=cq06_d[:])

            for b_rep in range(BPC * reps):
                b = b_rep % BPC
                # -------- key phase: e_pre for all 64 tiles --------
                pe = ps_e.tile([128, TILES * H], f32)  # [n, (T,h)] scores
                for g in range(GROUPS):
                    n0 = g * GT * 128
                    if dma_cast:
                        kbf = kload.tile([128, GT, QD], bf16, tag="kload")
                        nc.gpsimd.dma_start(
                            out=kbf[:],
                            in_=key_d[b, n0:n0 + GT * 128, :].rearrange(
                                "(t p) q -> p t q", p=128
                            ),
                        )
                    else:
                        kf = kload.tile([128, GT, QD], f32, tag="kload")
                        nc.sync.dma_start(
                            out=kf[:],
                            in_=key_d[b, n0:n0 + GT * 128, :].rearrange(
                                "(t p) q -> p t q", p=128
                            ),
                        )
                        kbf = kload.tile([128, GT, QD], bf16, tag="kcast")
                        nc.vector.tensor_copy(kbf[:], kf[:])

                    # transpose key tiles into [qd, n] chunks
                    ktc = []
                    for c in range(2):
                        kt = keyt.tile([128, GT * 128], bf16, tag="keyt")
                        ktc.append(kt)
                    for hh in range(GT // 4):  # half-groups of 4 tiles
                        for c in range(2):
                            pt = ps_t.tile([128, 512], bf16, tag="ps_t")
                            for tt in range(4):
                                t = hh * 4 + tt
                                nc.tensor.transpose(
                                    pt[:, tt * 128:(tt + 1) * 128],
                                    kbf[:, t, c * 128:(c + 1) * 128],
                                    ident_sb[:],
                                )
                            nc.vector.tensor_copy(
                                ktc[c][:, hh * 512:(hh + 1) * 512], pt[:]
                            )

                    # hk' = Wq.T @ keyT ; lrelu(hk' + qb) -> t_bf [hd, n]
                    tbf = []
                    for m in range(2):
                        tm = tlr.tile([128, GT * 128], bf16, tag="tlr")
                        tbf.append(tm)
                    for half in range(2):
                        cols = slice(half * 512, (half + 1) * 512)
                        for m in range(2):
                            ph = ps_hk.tile([128, 512], f32, tag="ps_hk")
                            for c in range(2):
                                nc.tensor.matmul(
                                    ph[:],
                                    wq_sb[:, (c * 2 + m) * 128:(c * 2 + m + 1) * 128],
                                    ktc[c][:, cols],
                                    start=(c == 0),
                                    stop=(c == 1),
                                )
                            if act_mode == "lrelu":
                                nc.scalar.activation(
                                    tbf[m][:, cols],
                                    ph[:],
                                    AF.Lrelu,
                                    bias=qb_sb[:, b * 2 + m:b * 2 + m + 1],
                                    scale=1.0,
                                    alpha=NEG_SLOPE,
                                )
                            else:
                                # |hk' + qb|; the 0.6*(hk'+qb) linear part is
                                # reconstituted via cq06 (softmax is shift-
                                # invariant so the qb part of it is dropped).
                                nc.scalar.activation(
                                    tbf[m][:, cols],
                                    ph[:],
                                    AF.Abs,
                                    bias=qb_sb[:, b * 2 + m:b * 2 + m + 1],
                                    scale=1.0,
                                )

                    # a-dot: e_pre[n, (T,h)] slices
                    for t in range(GT):
                        T = g * GT + t
                        if act_mode == "lrelu":
                            for m in range(2):
                                nc.tensor.matmul(
                                    pe[:, T * H:(T + 1) * H],
                                    tbf[m][:, t * 128:(t + 1) * 128],
                                    ablk_sb[:, m * H:(m + 1) * H],
                                    start=(m == 0),
                                    stop=(m == 1),
                                )
                        else:
                            # 0.4*|y| . a  +  0.6*y . a (via key @ (Wq@0.6a))
                            for m in range(2):
                                nc.tensor.matmul(
                                    pe[:, T * H:(T + 1) * H],
                                    tbf[m][:, t * 128:(t + 1) * 128],
                                    ablk_sb[:, m * H:(m + 1) * H],
                                    start=(m == 0),
                                    stop=False,
                                )
                            for c in range(2):
                                nc.tensor.matmul(
                                    pe[:, T * H:(T + 1) * H],
                                    ktc[c][:, t * 128:(t + 1) * 128],
                                    cq06_sb[:, c * H:(c + 1) * H],
                                    start=False,
                                    stop=(c == 1),
                                )

                # -------- softmax numerator (unnormalized) --------
                w_u = wbuf.tile([128, TILES * H], f32r, tag="wbuf")
                nc.scalar.activation(w_u[:], pe[:], AF.Exp)

                # e output: unnormalized exp weights (host divides by sum)
                nc.sync.dma_start(
                    out=e_d[b].rearrange("(T p) h -> p T h", p=128),
                    in_=w_u[:],
                )

                # -------- value phase: s[h,v] = sum_n w[n,h] * value[n,v] ----
                psc = ps_sc.tile([H, VD], f32, tag="ps_sc")
                for g in range(GROUPS):
                    n0 = g * GT * 128
                    vf = vload.tile([128, GT, VD], f32r, tag="vload")
                    nc.sync.dma_start(
                        out=vf[:],
                        in_=val_d[b, n0:n0 + GT * 128, :].rearrange(
                            "(t p) q -> p t q", p=128
                        ),
                    )
                    for t in range(GT):
                        T = g * GT + t
                        nc.tensor.matmul(
                            psc[:],
                            w_u[:, T * H:(T + 1) * H],
                            vf[:, t, :],
                            start=(T == 0),
                            stop=(T == TILES - 1),
                        )
                s_sb = sout.tile([H, VD], f32, tag="sout")
                nc.vector.tensor_copy(s_sb[:], psc[:])
                nc.sync.dma_start(out=s_d[b], in_=s_sb[:])

    nc.compile()
    return nc


ACT_MODE = "abs"   # HW Lrelu LUT ignores alpha (applies slope 0.01)
DMA_CAST = True


def _get_program():
    if "nc" not in _CACHE:
        _CACHE["nc"] = _build_program(dma_cast=DMA_CAST, act_mode=ACT_MODE)
    return _CACHE["nc"]


def _host_prep(query, key, value, Wq, bq, a, act_mode=None):
    """Build per-core input maps."""
    if act_mode is None:
        act_mode = ACT_MODE
    bf16 = ml_dtypes.bfloat16
    qb_all = (query.astype(np.float32) @ Wq + 2.0 * bq).astype(np.float32)  # [B, 256]
    wq_host = np.ascontiguousarray(
        Wq.reshape(2, 128, 2, 128).transpose(1, 0, 2, 3).reshape(128, 512)
    ).astype(bf16)
    ablk_np = np.zeros((HD, H), np.float32)
    for h in range(H):
        ablk_np[h * D:(h + 1) * D, h] = a
    a_scale = 1.0 if act_mode == "lrelu" else 0.4
    ablk_host = np.ascontiguousarray(
        (a_scale * ablk_np).reshape(2, 128, H).transpose(1, 0, 2).reshape(128, 8)
    ).astype(bf16)
    ident = np.eye(128, dtype=np.float32).astype(bf16)
    if act_mode == "abs":
        cq06_np = Wq.astype(np.float32) @ (0.6 * ablk_np)  # [256, 4]
        cq06_host = np.ascontiguousarray(
            cq06_np.reshape(2, 128, H).transpose(1, 0, 2).reshape(128, 8)
        ).astype(bf16)

    in_maps = []
    for c in range(NCORES):
        bs = slice(c * BPC, (c + 1) * BPC)
        qb_core = np.ascontiguousarray(
            qb_all[bs].reshape(BPC, 2, 128).transpose(2, 0, 1).reshape(128, BPC * 2)
        )
        m = {
            "key_c": np.ascontiguousarray(key[bs]).astype(np.float32),
            "val_c": np.ascontiguousarray(value[bs]).astype(np.float32),
            "qb_c": qb_core,
            "wq_c": wq_host,
            "ablk_c": ablk_host,
            "ident_c": ident,
        }
        if act_mode == "abs":
            m["cq06_c"] = cq06_host
        in_maps.append(m)
    return in_maps


def _host_post(results, Wv, bv):
    e_u = np.concatenate([r["e_c"] for r in results], axis=0)   # [B, N, H]
    s_u = np.concatenate([r["s_c"] for r in results], axis=0)   # [B, H, VD]
    tot = e_u.sum(axis=1)                                       # [B, H]
    e = e_u / tot[:, None, :]
    s = s_u / tot[:, :, None]
    scored = np.einsum("bhv,vhf->bhf", s, Wv.reshape(VD, H, D)) + bv.reshape(H, D)
    h_prime = np.maximum(scored.reshape(B, H * D), 0.0).astype(np.float32)
    return h_prime, e.astype(np.float32)


def kernel(query, key, value, Wq, bq, Wv, bv, a):
    from concourse.bass_utils import run_bass_kernel_spmd

    nc = _get_program()
    in_maps = _host_prep(
        np.asarray(query), np.asarray(key), np.asarray(value),
        np.asarray(Wq), np.asarray(bq), np.asarray(a),
    )
    out = run_bass_kernel_spmd(nc, in_maps, core_ids=list(range(NCORES)))
    h_prime, e = _host_post(out.results, np.asarray(Wv), np.asarray(bv))
    return h_prime, e
